# revision 59
# baseline (speedup 1.0000x reference)
"""Trainium2 Bass kernel for nn_KernelFilter_S (dynamic per-sample filter CNN).

Data-parallel over batch B=8 across 8 NeuronCores (one sample per core).

Per-core math (x = content[b], s = style[b]):
  c1 = conv3x3(x, ds_w) + ds_b                       [32,64,64]
  pooled_F = mean_HW(conv3x3(s, cwF)) + cbF          [32]    (F = 1,2)
  filtF = (pooled_F @ fwF.T + fbF).reshape(32,32,3,3)
  c2 = leaky(conv3x3_dyn(c1, filt1), 0.2)
  c3 = conv3x3_dyn(c2, filt2)
  out = x + conv3x3(c3, up_w) + up_b                 [512,64,64]

Structure (final):
  * mean-pool-of-conv -> 9 rectangle sums R[c,t] per style channel,
    computed ON THE PE: style is provided pixel-partition transposed,
    col-tiled matmul waves against 0/1 rectangle masks, then 4 tiny
    K=9 transpose matmuls against a 9x9 identity restore the
    channel-partition layout. No DVE/Act whole-image reductions.
  * all convs as PSUM-accumulated matmuls over zero-padded 66x66 images,
    8 row-tiles of 8 rows -> N=512 via 2D access patterns; PE column
    tiling (4 concurrent 32-col matmuls) for ds waves / dyn taps / rect;
    dyn convs are tap-outer so the 4 col-group matmuls run concurrently.
  * up conv (K=97 kx-packed X3 + ones-row bias, no identity matmuls);
    residual added during the PSUM drain (DVE 1-op / Act+GpSimd 2-op
    split); rotates over all 4 psum pool groups.
  * FC in slot order: fw columns are host-permuted so each filter is a
    contiguous strided-partition block of the packed drain buffer -
    one [98,512]-drain per 4 col-group segments and a single SBUF DMA
    per filter (per-transfer latency ~1.4us makes DMA count critical).
  * zero-input warmup matmuls bridge input-arrival and filter-reshape
    latency so the PE HAM clock never drops mid-kernel.
  * DMA: per-ring HWDGE transfers are latency/trigger bound; the Act
    ring only carries transfers triggered before Act compute starts
    (trigger instructions queue behind 680ns drain ops); all bulk
    streams on the SP ring in deadline order; x3 replicate bands split
    across both rings; eighth/half-chunk output stores overlap the up
    conv so only the last ~1us of stores trails the final drain.
"""

import os
import sys
import numpy as np

sys.path.insert(0, "/opt/trn_rl_repo")

import concourse.bass as bass
import concourse.bacc as bacc
import concourse.mybir as mybir
import concourse.tile as tile
from concourse.bass_utils import run_bass_kernel_spmd

F32 = mybir.dt.float32
BF16 = mybir.dt.bfloat16
FP8 = mybir.dt.float8e4
NP_BF16 = np.dtype(mybir.dt.np(BF16))
NP_FP8 = np.dtype(mybir.dt.np(FP8))

H = W = 64
PW = W + 2              # padded row width = 66
NPIX = H * W            # 4096
NPAD = (H + 2) * PW     # 66*66 = 4356
CIN = 512
INNER = 32
NCH = CIN // 128        # 4
NRT = 8                 # row tiles
TR = 8                  # rows per tile
NT = TR * W             # 512 = psum bank
ADD = mybir.AluOpType.add
MUL = mybir.AluOpType.mult


def _yx(ap):
    return ap.rearrange("p (y x) -> p y x", x=PW)


def _build_program():
    nc = bacc.Bacc(None, target_bir_lowering=False)

    content_h = nc.dram_tensor("content", [CIN, NPAD], FP8, kind="ExternalInput")
    contf_h = nc.dram_tensor("contf", [CIN, NPIX], BF16, kind="ExternalInput")
    stT_h = nc.dram_tensor("style_T", [128, 32 * 512], FP8, kind="ExternalInput")
    mask_h = nc.dram_tensor("masks", [128, 32 * 9], FP8, kind="ExternalInput")
    id9_h = nc.dram_tensor("id9", [9, 9], BF16, kind="ExternalInput")
    w_ds_h = nc.dram_tensor("w_ds", [128, 36 * INNER], FP8, kind="ExternalInput")
    cw_h = nc.dram_tensor("cw", [128, 36 * 64], BF16, kind="ExternalInput")
    w_up_h = nc.dram_tensor("w_up", [97, 2304], BF16, kind="ExternalInput")
    fw_h = nc.dram_tensor("fw", [66, 9216], FP8, kind="ExternalInput")
    dsb_h = nc.dram_tensor("ds_b", [INNER], F32, kind="ExternalInput")
    cb_h = nc.dram_tensor("cb", [64], F32, kind="ExternalInput")
    lfi_h = nc.dram_tensor("lfinit", [66, 2], FP8, kind="ExternalInput")
    out_h = nc.dram_tensor("out", [CIN, NPIX], BF16, kind="ExternalOutput")

    with tile.TileContext(nc) as tc:
        with (
            tc.tile_pool(name="const", bufs=1) as const,
            tc.tile_pool(name="img", bufs=1) as img,
            tc.tile_pool(name="sx", bufs=4) as sx,
            tc.tile_pool(name="drain", bufs=4) as drain,
            tc.tile_pool(name="cps", bufs=3, space=bass.MemorySpace.PSUM) as cps,
            tc.tile_pool(name="dps", bufs=3, space=bass.MemorySpace.PSUM) as dps,
            tc.tile_pool(name="pps", bufs=1, space=bass.MemorySpace.PSUM) as pps_pool,
            tc.tile_pool(name="fps", bufs=1, space=bass.MemorySpace.PSUM) as fps_pool,
        ):
            # ---- PE warm-up on zeros (no DMA dependency; gpsimd boots
            # earliest so the memset lands asap) --------------------------
            wz = const.tile([128, NT], BF16, tag="wz")
            nc.gpsimd.memset(wz[:], 0.0)
            warm_ps = cps.tile([128, NT], F32, tag="cps")
            for i in range(26):
                nc.tensor.matmul(
                    warm_ps[0:32], wz[:, 0:32], wz[:], start=True, stop=True
                )

            # ---- DMA triggers (HWDGE rings are FIFO; order = priority) ---
            w_ds_sb = const.tile([128, 36 * INNER], FP8, tag="wds")
            nc.sync.dma_start(out=w_ds_sb[:], in_=w_ds_h[:])
            cpad = [None] * NCH

            def load_cpad(c, eng):
                cp = img.tile([128, NPAD], FP8, tag=f"cpad{c}")
                cpad[c] = cp
                eng.dma_start(
                    out=cp[:],
                    in_=content_h[:].rearrange("(c p) q -> c p q", p=128)[c],
                )

            stT = []

            def load_stT(hf, eng):
                st = img.tile([128, 16 * 512], FP8, tag=f"stT{hf}")
                eng.dma_start(
                    out=st[:], in_=stT_h[:, hf * 8192:(hf + 1) * 8192]
                )
                stT.append(st)

            # scalar(Act) ring only gets transfers triggered before Act
            # compute begins - trigger instructions sit in the Act FIFO
            # behind 680ns drain ops, which starves the ring mid-kernel.
            # Everything else streams on the sync(SP) ring in deadline
            # order; SP has no compute so its triggers always flow.
            load_cpad(0, nc.scalar)
            load_cpad(1, nc.sync)
            load_cpad(2, nc.scalar)
            load_cpad(3, nc.sync)
            load_stT(0, nc.scalar)
            load_stT(1, nc.sync)
            cw_sb = const.tile([128, 36 * 64], BF16, tag="cw")
            nc.sync.dma_start(out=cw_sb[:], in_=cw_h[:])
            fw_sb = const.tile([66, 9216], FP8, tag="fw")
            nc.sync.dma_start(out=fw_sb[:], in_=fw_h[:])
            w_up_sb = const.tile([97, 2304], BF16, tag="wup")
            nc.sync.dma_start(out=w_up_sb[:], in_=w_up_h[:])
            cont = [None] * NCH
            for c in range(NCH):
                ct = img.tile([128, NPIX], BF16, tag=f"cont{c}")
                nc.sync.dma_start(
                    out=ct[:], in_=contf_h[:].rearrange("(c p) q -> c p q", p=128)[c]
                )
                cont[c] = ct
            # tiny constants first on the gpsimd (SWDGE) ring
            mask_sb = const.tile([128, 32 * 9], FP8, tag="masks")
            nc.gpsimd.dma_start(out=mask_sb[:], in_=mask_h[:])
            id9_sb = const.tile([9, 9], BF16, tag="id9")
            nc.gpsimd.dma_start(out=id9_sb[:], in_=id9_h[:])
            dsb_sb = const.tile([INNER, 1], F32, tag="dsb")
            nc.gpsimd.dma_start(out=dsb_sb[:], in_=dsb_h[:].rearrange("(o u) -> o u", u=1))
            cb_sb = const.tile([64, 1], F32, tag="cb")
            nc.gpsimd.dma_start(out=cb_sb[:], in_=cb_h[:].rearrange("(o u) -> o u", u=1))
            lf = const.tile([66, 2], FP8, tag="lf")
            nc.gpsimd.dma_start(out=lf[:], in_=lfi_h[:])

            def warm(n):
                for i in range(n):
                    nc.tensor.matmul(
                        warm_ps[0:32], wz[:, 0:32], wz[:], start=True, stop=True
                    )

            # ---- padded intermediates: border memsets --------------------
            def border_memsets(t, eng1, eng2, wide):
                hw = wide // 2
                eng1.memset(t[:, 0:PW + hw], 0.0)
                off = PW - hw
                eng2.memset(
                    t[:, off:off + 65 * PW]
                    .rearrange("p (a b) -> p a b", b=PW)[:, :, 0:wide],
                    0.0,
                )
                eng1.memset(t[:, NPAD - PW - hw:NPAD], 0.0)

            c1pad = img.tile([32, NPAD], BF16, tag="c1pad")
            border_memsets(c1pad, nc.vector, nc.gpsimd, 2)
            c2pad = img.tile([32, NPAD], BF16, tag="c2pad")
            border_memsets(c2pad, nc.vector, nc.gpsimd, 2)
            x3c3 = img.tile([97, NPAD], BF16, tag="x3c3")
            border_memsets(x3c3[0:96], nc.vector, nc.gpsimd, 4)
            nc.gpsimd.memset(x3c3[96:97, :], 1.0)   # ones row (up-conv bias)

            # ---- ds conv ------------------------------------------------
            def ds_tile(rt):
                pool = cps if rt % 2 == 0 else dps
                ps = pool.tile([128, NT], F32, tag=pool.name)
                psr = ps[:].rearrange("p (r x) -> p r x", x=W)
                for w in range(9):
                    for g in range(4):
                        jj = w * 4 + g
                        c, t = divmod(jj, 9)
                        ky, kx = divmod(t, 3)
                        rhs = _yx(cpad[c][:])[:, rt * TR + ky:rt * TR + ky + TR,
                                              kx:kx + W]
                        nc.tensor.matmul(
                            psr[32 * g:32 * g + 32],
                            w_ds_sb[:, jj * 32:(jj + 1) * 32],
                            rhs,
                            start=(w == 0), stop=(w == 8),
                            tile_position=(0, 32 * g),
                        )
                # drain: sum 4 col-group partials + bias -> c1pad center
                # (PSUM readable only by DVE/Act, max one PSUM operand each)
                s1 = drain.tile([32, NT], F32, tag="s1")
                nc.scalar.activation(
                    s1[:], psr[32:64], mybir.ActivationFunctionType.Copy
                )
                s2 = drain.tile([32, NT], F32, tag="s2")
                nc.scalar.activation(
                    s2[:], psr[64:96], mybir.ActivationFunctionType.Copy
                )
                a1 = drain.tile([32, NT], F32, tag="a1")
                nc.vector.tensor_add(a1[:], psr[0:32], s1[:])
                b1 = drain.tile([32, NT], F32, tag="b1")
                nc.vector.scalar_tensor_tensor(
                    b1[:], psr[96:128], dsb_sb[:], s2[:], op0=ADD, op1=ADD
                )
                rows = slice(rt * TR + 1, rt * TR + 1 + TR)
                ctr = _yx(c1pad[:])[:, rows, 1:1 + W]
                nc.gpsimd.tensor_add(
                    ctr, a1[:].rearrange("p (r x) -> p r x", x=W),
                    b1[:].rearrange("p (r x) -> p r x", x=W),
                )

            def x3_replicate(dst, a, b):
                # duplicate center group into kx=0 / kx=2 shifted groups on
                # the sync ring (idle from here to the output stores).
                lo, hi = a * PW, b * PW
                src = dst[32:64, lo:hi]
                nc.scalar.dma_start(out=dst[0:32, lo + 1:hi + 1], in_=src)
                nc.sync.dma_start(out=dst[64:96, lo - 1:hi - 1], in_=src)

            for rt in range(5):
                ds_tile(rt)

            # ---- rect sums on the PE: R_T[t, c] = mask_t . style_T -------
            rt_ps = pps_pool.tile([128, NT], F32, tag="pooled")
            for w in range(8):
                for g in range(4):
                    j = w * 4 + g
                    rhs = stT[j // 16][:, (j % 16) * 512:(j % 16) * 512 + 512]
                    nc.tensor.matmul(
                        rt_ps[32 * g:32 * g + 9],
                        mask_sb[:, j * 9:(j + 1) * 9], rhs,
                        start=(w == 0), stop=(w == 7),
                        tile_position=(0, 32 * g),
                    )
            # drain 4 col-group partials -> rtb [9, 512] bf16
            rs1 = drain.tile([9, NT], F32, tag="s1")
            nc.scalar.activation(
                rs1[:], rt_ps[32:41], mybir.ActivationFunctionType.Copy
            )
            rs2 = drain.tile([9, NT], F32, tag="s2")
            nc.scalar.activation(
                rs2[:], rt_ps[96:105], mybir.ActivationFunctionType.Copy
            )
            ra = drain.tile([9, NT], F32, tag="a1")
            nc.vector.tensor_add(ra[:], rt_ps[0:9], rs1[:])
            rb = drain.tile([9, NT], F32, tag="b1")
            nc.vector.tensor_add(rb[:], rt_ps[64:73], rs2[:])
            rtb = const.tile([9, NT], BF16, tag="rtb")
            nc.gpsimd.tensor_add(rtb[:], ra[:], rb[:])


            ds_tile(5)
            ds_tile(6)

            # ---- transpose R_T -> Rb[c, chunk*9+t] via K=9 matmuls -------
            tps = fps_pool.tile([128, 36], F32, tag="fc")
            for ch in range(NCH):
                nc.tensor.matmul(
                    tps[:, ch * 9:(ch + 1) * 9],
                    rtb[:, ch * 128:(ch + 1) * 128], id9_sb[:],
                    start=True, stop=True,
                )
            Rb = const.tile([128, 36], BF16, tag="Rb")
            nc.vector.tensor_copy(Rb[:], tps[:])

            ds_tile(7)

            # ---- predictor: pooled -> lf (DVE+Act in parallel) -----------
            pps = pps_pool.tile([64, 1], F32, tag="pooled")
            for jj in range(36):
                nc.tensor.matmul(
                    pps[:], cw_sb[:, jj * 64:(jj + 1) * 64], Rb[:, jj:jj + 1],
                    start=(jj == 0), stop=(jj == 35),
                )
            # pooled is scaled by 64 so fp8 holds it with full precision;
            # fw rows are host-scaled by 64; FC drain divides by 4096.
            nc.vector.scalar_tensor_tensor(
                lf[0:32, 0:1], pps[0:32], 64.0 / NPIX, cb_sb[0:32],
                op0=MUL, op1=ADD,
            )
            nc.scalar.activation(
                lf[32:64, 1:2], pps[32:64], mybir.ActivationFunctionType.Identity,
                bias=cb_sb[32:64], scale=64.0 / NPIX,
            )

            # ---- FC: slot-ordered output --------------------------------
            # fw columns are host-permuted so that filter F is exactly the
            # strided-partition block fcT[F::32, 0:2304] - a single DMA per
            # filter, no de-interleave chain (per-transfer latency ~1.4us
            # makes many small DMAs the critical path otherwise).
            fcT = const.tile([98, 2304], BF16, tag="fcT")
            for w in range(5):
                pool = (cps, dps)[w % 2]
                pw = pool.tile([128, NT], F32, tag=pool.name)
                ncols = 512 if w < 4 else 256
                for a in range(4):
                    off = w * 2048 + a * 512 if w < 4 else 8192 + a * 256
                    nc.tensor.matmul(
                        pw[32 * a:32 * a + 2, 0:ncols], lf[:],
                        fw_sb[:, off:off + ncols],
                        start=True, stop=True, tile_position=(0, 32 * a),
                    )
                dst = fcT[:, w * 512:w * 512 + ncols]
                if w % 2 == 0:
                    nc.vector.tensor_scalar_mul(dst, pw[0:98, 0:ncols],
                                                1.0 / NPIX)
                else:
                    nc.scalar.activation(
                        dst, pw[0:98, 0:ncols],
                        mybir.ActivationFunctionType.Identity, scale=1.0 / NPIX,
                    )
            filt = []
            for F in range(2):
                ft = const.tile([32, 288], BF16, tag=f"filt{F}")
                # ft0 gates dyn1: use the SWDGE ring whose trigger queue is
                # idle here (Act-ring triggers wait behind 680ns drain ops)
                eng = nc.gpsimd if F == 0 else nc.scalar
                eng.dma_start(out=ft[:], in_=fcT[F:98:32, 0:2304])
                filt.append(ft)
            # zero-input warmups bridge the filter-reshape latency
            for i in range(36):
                nc.tensor.matmul(
                    warm_ps[0:32], wz[:, 0:32], wz[:], start=True, stop=True
                )

            # ---- dyn convs: tap-outer waves, 4 row-tiles concurrent ------
            def dyn_conv(src, f, write_out, sws=((0, 1, 2, 3), (4, 5, 6, 7))):
                for si, rts in enumerate(sws):
                    if si:
                        warm(2)   # keep PE duty high: HAM re-throttles on
                                  # micro-idles between wave bursts
                    ps = dps.tile([128, NT], F32, tag="dps")
                    psr = ps[:].rearrange("p (r x) -> p r x", x=W)
                    for t in range(9):
                        ky, kx = divmod(t, 3)
                        for g, rt in enumerate(rts):
                            rhs = _yx(src[:])[:, rt * TR + ky:rt * TR + ky + TR,
                                              kx:kx + W]
                            nc.tensor.matmul(
                                psr[32 * g:32 * g + 32],
                                f[:, t * 32:(t + 1) * 32], rhs,
                                start=(t == 0), stop=(t == 8),
                                tile_position=(0, 32 * g),
                            )
                    for g, rt in enumerate(rts):
                        write_out(rt, psr[32 * g:32 * g + 32])

            def dyn1_out(rt, psr):
                # split leaky drains Act/DVE: the serial Act-only chain
                # (8 x 680ns) gated dyn2's first wave; DVE is idle here
                rows = slice(rt * TR + 1, rt * TR + 1 + TR)
                dst = _yx(c2pad[:])[:, rows, 1:1 + W]
                if rt % 2 == 0:
                    nc.scalar.activation(
                        dst, psr, mybir.ActivationFunctionType.Lrelu, alpha=0.2,
                    )
                else:
                    lt = drain.tile([32, NT], F32, tag="s1")
                    nc.vector.tensor_copy(lt[:], psr)
                    nc.vector.scalar_tensor_tensor(
                        dst, lt[:].rearrange("p (r x) -> p r x", x=W), 0.2,
                        lt[:].rearrange("p (r x) -> p r x", x=W),
                        op0=MUL, op1=mybir.AluOpType.max,
                    )

            def dyn2_out(rt, psr):
                rows = slice(rt * TR + 1, rt * TR + 1 + TR)
                ctr = _yx(x3c3[32:64])[:, rows, 1:1 + W]
                if rt % 2 == 0:
                    nc.scalar.activation(
                        ctr, psr, mybir.ActivationFunctionType.Copy
                    )
                else:
                    nc.vector.tensor_copy(ctr, psr)
                    # replicate the finished 16-row band right away so the
                    # up conv never waits on the kx-shifted copies
                    x3_replicate(x3c3, rt * TR - 7, rt * TR + 9)

            dyn_conv(c1pad, filt[0], dyn1_out)
            warm(2)
            dyn_conv(c2pad, filt[1], dyn2_out)
            warm(3)

            # ---- up conv + residual (bias via ones row, no identity) -----
            up_pools = (cps, dps, fps_pool, pps_pool)
            for cc in range(NCH):
                outt = sx.tile([128, NPIX], BF16, tag="sx")
                oh = out_h[:].rearrange("(c p) q -> c p q", p=128)[cc]
                for rt in range(NRT):
                    idx = cc * NRT + rt
                    pool = up_pools[idx % 4]
                    ps = pool.tile([128, NT], F32,
                                   tag={"cps": "cps", "dps": "dps"}.get(
                                       pool.name, "fc" if pool is fps_pool
                                       else "pooled"))
                    psr = ps[:].rearrange("p (r x) -> p r x", x=W)
                    for ky in range(3):
                        rhs = _yx(x3c3[0:97])[:, rt * TR + ky:rt * TR + ky + TR,
                                              1:1 + W]
                        nc.tensor.matmul(
                            psr, w_up_sb[:, (ky * NCH + cc) * 128:
                                         (ky * NCH + cc + 1) * 128], rhs,
                            start=(ky == 0), stop=(ky == 2),
                        )
                    oseg = outt[:, rt * NT:(rt + 1) * NT]
                    cseg = cont[cc][:, rt * NT:(rt + 1) * NT]
                    # residual+scale in the drain (the conv path carries a
                    # x64 scale from w_ds; bias already in psum via ones
                    # row); 2/3 DVE : 1/3 Act+GpSimd balances engine time
                    if idx % 4 != 3 or idx >= 24:
                        nc.vector.scalar_tensor_tensor(
                            oseg, ps[:], 1.0 / 64.0, cseg, op0=MUL, op1=ADD
                        )
                    else:
                        ut = drain.tile([128, NT], BF16, tag="ut")
                        nc.scalar.activation(
                            ut[:], ps[:], mybir.ActivationFunctionType.Identity,
                            scale=1.0 / 64.0,
                        )
                        nc.gpsimd.tensor_add(oseg, ut[:], cseg)
                    # ship half-chunks; the final half goes as two quarter
                    # stores on both rings to shorten the tail
                    if rt == 3 or rt == 7:
                        hb = rt // 4
                        if cc == NCH - 1 and hb == 1:
                            # final half as four eighth-stores on both rings
                            # so the tail after the last drain is minimal
                            for i8, oq in ((4, nc.scalar), (5, nc.sync),
                                           (6, nc.sync), (7, nc.sync)):
                                oq.dma_start(
                                    out=oh[:, i8 * 512:(i8 + 1) * 512],
                                    in_=outt[:, i8 * 512:(i8 + 1) * 512],
                                )
                        else:
                            oq = nc.sync if (cc * 2 + hb) % 2 == 0 else nc.scalar
                            oq.dma_start(
                                out=oh[:, hb * 2048:(hb + 1) * 2048],
                                in_=outt[:, hb * 2048:(hb + 1) * 2048],
                            )

    nc.compile()
    return nc


_NC_CACHE = None


def _get_nc():
    global _NC_CACHE
    if _NC_CACHE is None:
        _NC_CACHE = _build_program()
    return _NC_CACHE


def _make_masks():
    # mask[p, j*9+t] = 1 if pixel (j*128+p) lies in the valid-overlap
    # rectangle of tap t = ky*3+kx for SAME 3x3 conv + global mean pool
    px = np.arange(NPIX)
    y, x = px // W, px % W
    m = np.zeros((NPIX, 9), np.float32)
    for t in range(9):
        ky, kx = divmod(t, 3)
        ys = (y >= max(0, ky - 1)) & (y < H - max(0, 1 - ky))
        xs = (x >= max(0, kx - 1)) & (x < W - max(0, 1 - kx))
        m[:, t] = (ys & xs).astype(np.float32)
    # [px, t] -> [p, j*9+t]
    return np.ascontiguousarray(
        m.reshape(32, 128, 9).transpose(1, 0, 2).reshape(128, 32 * 9)
    ).astype(NP_FP8)


_MASKS = _make_masks()
_ID9 = np.eye(9, dtype=np.float32).astype(NP_BF16)


def _prep_weights(ds_w, up_w, up_b, f1_cw, f2_cw, f1_fw, f2_fw, f1_fb, f2_fb):
    # w_ds block jj = c*9 + t: [p, o] = 64 * ds_w[o, c*128+p, t]  (fp8)
    X = ds_w.transpose(1, 2, 3, 0).reshape(CIN, 9, INNER) * 64.0   # (i, t, o)
    w_ds = np.ascontiguousarray(
        X.reshape(NCH, 128, 9, INNER).transpose(1, 0, 2, 3).reshape(128, 36 * INNER)
    ).astype(NP_FP8)
    # cw block jj = c*9 + t: [p, F*32+o] = fF_cw[o, c*128+p, t]
    cws = []
    for cw in (f1_cw, f2_cw):
        Y = cw.transpose(1, 2, 3, 0).reshape(CIN, 9, INNER)
        cws.append(Y.reshape(NCH, 128, 9, INNER).transpose(1, 0, 2, 3))
    cwm = np.concatenate(cws, axis=3).reshape(128, 36 * 64)        # [p,(c,t),(F,o)]
    cwm = np.ascontiguousarray(cwm).astype(NP_BF16)
    # w_up [kx*32+i, (ky*4+cc)*128+oc] = up_w[cc*128+oc, i, ky, kx]
    # row 96 (ones row in X3) = 64*up_b in the ky=1 blocks (bias fold)
    B = up_w.reshape(NCH, 128, INNER, 3, 3).transpose(4, 2, 3, 0, 1)
    w_up = np.zeros((97, 2304), np.float32)
    w_up[0:96, 0:1536] = B.reshape(96, 3 * NCH * 128)
    w_up[96, 0:1536].reshape(3, NCH * 128)[1] = 64.0 * up_b
    w_up = np.ascontiguousarray(w_up).astype(NP_BF16)
    # fw rows: 0-31 f1, 32-63 f2, 64 fb1, 65 fb2; col n = ((kx*32+i)*3+ky)*32+o
    def permw(fw):
        # fw [(o,i,ky,kx), k] -> [k, (i,ky,kx,o)]
        Z = fw.reshape(INNER, INNER, 3, 3, INNER).transpose(1, 2, 3, 0, 4)
        return Z.reshape(9216, INNER).T

    def permb(fb):
        return fb.reshape(INNER, INNER, 3, 3).transpose(1, 2, 3, 0).reshape(9216)

    # fw rows x64 / bias rows x4096 so fp8 storage keeps precision;
    # the on-chip FC drain divides by 4096 (lf carries pooled x64).
    fwm = np.zeros((66, 9216), np.float32)
    fwm[0:32] = permw(f1_fw) * 64.0
    fwm[32:64] = permw(f2_fw) * 64.0
    fwm[64] = permb(f1_fb) * 4096.0
    fwm[65] = permb(f2_fb) * 4096.0
    # slot order: FC wave/group (w,a) computes filter elements
    # m = a*2304 + w*512 + r so filter F = fcT[F::32, 0:2304] contiguous
    pos_to_m = np.empty(9216, np.int64)
    for w in range(4):
        for a in range(4):
            p = w * 2048 + a * 512
            pos_to_m[p:p + 512] = a * 2304 + w * 512 + np.arange(512)
    for a in range(4):
        p = 8192 + a * 256
        pos_to_m[p:p + 256] = a * 2304 + 2048 + np.arange(256)
    fwm = np.ascontiguousarray(fwm[:, pos_to_m]).astype(NP_FP8)
    return w_ds, cwm, w_up, fwm


def kernel(content, style, ds_w, ds_b, up_w, up_b,
           f1_cw, f1_cb, f1_fw, f1_fb,
           f2_cw, f2_cb, f2_fw, f2_fb):
    content = np.asarray(content, np.float32)
    style = np.asarray(style, np.float32)
    B = content.shape[0]
    assert B == 8

    w_ds, cwm, w_up, fwm = _prep_weights(
        np.asarray(ds_w, np.float32), np.asarray(up_w, np.float32),
        np.asarray(up_b, np.float32),
        np.asarray(f1_cw, np.float32), np.asarray(f2_cw, np.float32),
        np.asarray(f1_fw, np.float32), np.asarray(f2_fw, np.float32),
        np.asarray(f1_fb, np.float32), np.asarray(f2_fb, np.float32))
    cb = np.concatenate([np.asarray(f1_cb, np.float32),
                         np.asarray(f2_cb, np.float32)]) * 64.0

    lfi = np.zeros((66, 2), np.float32)
    lfi[64, 0] = 1.0
    lfi[65, 1] = 1.0
    shared = {
        "w_ds": w_ds, "cw": cwm, "w_up": w_up, "fw": fwm,
        "ds_b": np.asarray(ds_b, np.float32) * 64.0,
        "cb": cb, "lfinit": lfi.astype(NP_FP8),
        "masks": _MASKS, "id9": _ID9,
    }
    cont_pad = np.zeros((B, CIN, H + 2, PW), NP_FP8)
    cont_pad[:, :, 1:65, 1:65] = content.reshape(B, CIN, H, W).astype(NP_FP8)
    cont_pad = cont_pad.reshape(B, CIN, NPAD)
    cont_bf = content.reshape(B, CIN, NPIX).astype(NP_BF16)
    # style transposed to pixel-partition layout [p, j*512 + ch]
    S8 = style.reshape(B, CIN, NPIX).astype(NP_FP8)
    styl_T = np.ascontiguousarray(
        S8.transpose(0, 2, 1).reshape(B, 32, 128, CIN)
        .transpose(0, 2, 1, 3).reshape(B, 128, 32 * CIN)
    )
    in_maps = []
    for b in range(B):
        m = dict(shared)
        m["content"] = np.ascontiguousarray(cont_pad[b])
        m["contf"] = np.ascontiguousarray(cont_bf[b])
        m["style_T"] = styl_T[b]
        in_maps.append(m)

    nc = _get_nc()
    trace = bool(int(os.environ.get("KF_TRACE", "0")))
    res = run_bass_kernel_spmd(nc, in_maps, core_ids=list(range(B)), trace=trace)
    if trace and getattr(res, "exec_time_ns", None) is not None:
        print(f"HW exec time: {res.exec_time_ns} ns")
        kernel.last_exec_ns = res.exec_time_ns
    kernel.last_results = res
    out = np.stack([res.results[b]["out"].reshape(CIN, H, W) for b in range(B)])
    return out.astype(np.float32)


if __name__ == "__main__":
    _get_nc()
    print("program built + compiled OK")
